# revision 4
# baseline (speedup 1.0000x reference)
"""Trainium2 Bass kernel for nn_Dependency_GCNLayer (relational GCN layer).

Reference semantics:
    out = relu(x @ W_self.T + b_self
               + scatter_add over forward edges:  dep += W_rel[r]   @ x[gov] + b_rel[r]
               + scatter_add over reverse edges:  gov += W_rel[r+R] @ x[dep] + b_rel[r+R])

Strategy (8 cores, SPMD single program):
  - Output rows (nodes) sharded 2500/core. x + weights replicated.
  - Unified edge list (dest, src, rel) with reverse edges mapped to rel+R.
  - Host groups edges by (dest-block of 256 nodes, rel), pads each group to a
    multiple of 128 slots (shared caps across cores => identical instruction
    stream; per-core data differs only through input tensors).
  - Device, per dest-block:
      * dma_gather x[src] rows for the block's edge slots (SWDGE custom op)
      * per rel: one-hot aggregation  S_T[f, d] += G_chunk[e, f].T @ P[e, d]
        (P built on DVE via iota==dest_local), accumulated in PSUM
      * copy S_T -> SBUF, then dense transform out_T[o, d] += W_r.T[f, o].T @ S_T
        accumulated in PSUM across all rels + self term + (counts @ biases)
      * ReLU -> out_T in DRAM, host transposes/concats shards.
"""

import math

import numpy as np

import concourse.bacc as bacc
import concourse.mybir as mybir
import concourse.tile as tile
from concourse import library_config
from concourse.bass_utils import run_bass_kernel_spmd

# ---------------------------------------------------------------- constants
N, D, R, E = 20000, 256, 12, 200000
NCORES = 8
SHARD = N // NCORES          # 2500
BLK = 256                    # dest-node block width (= matmul N)
NBLK = math.ceil(SHARD / BLK)   # 10
SHARD_PAD = NBLK * BLK       # 2560
R2 = 2 * R                   # 24 relation slots (fwd + rev)
NW = R2 + 1                  # + self
NAUG = 20480                 # x rows padded with zero rows (int16-indexable)
ZROW = N                     # first zero row index
FP = mybir.dt.float32

_CACHE = {}


# ---------------------------------------------------------------- host prep
def _wrap_idx(idx_local: np.ndarray) -> np.ndarray:
    """Layout gather indices for dma_gather: slot s -> wrapped[ch, k] with
    ch = s % 16, k = (s // 128) * 8 + (s % 128) // 16 (measured on HW)."""
    n = idx_local.shape[0]
    w = np.zeros((16, n // 16), dtype=np.int16)
    s = np.arange(n)
    w[s % 16, (s // 128) * 8 + (s % 128) // 16] = idx_local.astype(np.int16)
    return np.tile(w, (8, 1))  # [128, n/16]


def _preprocess(x, dep_idx, rel_idx, gov_idx, W_self, b_self, W_rel, b_rel):
    dep = np.asarray(dep_idx).astype(np.int64)
    gov = np.asarray(gov_idx).astype(np.int64)
    rel = np.asarray(rel_idx).astype(np.int64)

    dest = np.concatenate([dep, gov])
    src = np.concatenate([gov, dep])
    rr = np.concatenate([rel, rel + R])

    core = dest // SHARD
    loc = dest % SHARD
    blk = loc // BLK
    dl = (loc % BLK).astype(np.float32)

    gid = (core * NBLK + blk) * R2 + rr           # group id
    counts = np.bincount(gid, minlength=NCORES * NBLK * R2)
    counts = counts.reshape(NCORES, NBLK, R2)
    caps = counts.max(axis=0)                     # [NBLK, R2]
    caps = ((caps + 127) // 128) * 128

    # slot offsets per (blk, rel), shared by all cores
    off = np.zeros((NBLK, R2), dtype=np.int64)
    off.flat[1:] = np.cumsum(caps.flat)[:-1]
    total = int(caps.sum())

    # rank of each edge within its (core, blk, rel) group
    order = np.argsort(gid, kind="stable")
    gs = gid[order]
    starts = np.searchsorted(gs, np.arange(NCORES * NBLK * R2))
    rank = np.arange(gs.shape[0]) - starts[gs]
    slot = off[blk[order], rr[order]] + rank      # slot within core's array
    core_o = core[order]

    idx_arr = np.full((NCORES, total), ZROW, dtype=np.int64)
    dl_arr = np.full((NCORES, total), -1.0, dtype=np.float32)
    idx_arr[core_o, slot] = src[order]
    dl_arr[core_o, slot] = dl[order]

    # gather calls: per (block, rel-half); all slot ranges 128-aligned
    halves = []
    for b in range(NBLK):
        for h in range(2):
            r0, r1 = h * (R2 // 2), (h + 1) * (R2 // 2)
            coff = int(off[b, r0])
            cap_h = int(caps[b, r0:r1].sum())
            halves.append((b, h, coff, cap_h))

    x = np.asarray(x, np.float32)
    x_aug = np.zeros((NAUG, D), dtype=np.float32)
    x_aug[:N] = x

    # per-core tensors
    W_rel = np.asarray(W_rel, np.float32)
    W_self = np.asarray(W_self, np.float32)
    b_rel = np.asarray(b_rel, np.float32)
    b_self = np.asarray(b_self, np.float32)

    Wt = np.zeros((128, 2, NW, D), dtype=np.float32)
    Wrt = W_rel.transpose(2, 0, 1).reshape(2, 128, R2, D)   # [fi, p, r, o]
    Wt[:, :, :R2, :] = Wrt.transpose(1, 0, 2, 3)
    Wst = W_self.T.reshape(2, 128, D)                       # [fi, p, o]
    Wt[:, :, R2, :] = Wst.transpose(1, 0, 2)

    B = np.zeros((NW, D), dtype=np.float32)
    B[:R2] = b_rel
    B[R2] = b_self

    iota = np.broadcast_to(np.arange(BLK, dtype=np.float32), (128, BLK)).copy()

    in_maps = []
    for c in range(NCORES):
        xs = x[c * SHARD:(c + 1) * SHARD]
        xT = np.zeros((128, 2, SHARD_PAD), dtype=np.float32)
        xT[:, :, :SHARD] = xs.T.reshape(2, 128, SHARD).transpose(1, 0, 2)

        lc = loc[core == c]
        rc = rr[core == c]
        C = np.zeros((NW, SHARD_PAD), dtype=np.float32)
        cc = np.bincount(lc * R2 + rc, minlength=SHARD * R2).reshape(SHARD, R2)
        C[:R2, :SHARD] = cc.T
        C[R2, :SHARD] = 1.0

        wrapped = np.concatenate(
            [_wrap_idx(idx_arr[c, coff:coff + cap_h])
             for (_, _, coff, cap_h) in halves], axis=1)
        dlt = dl_arr[c].reshape(-1, 128).T.copy()            # [128, total/128]

        in_maps.append({
            "x_aug": x_aug,
            "idxs": wrapped,
            "dl": dlt,
            "xT": xT,
            "C": C,
            "Wt": Wt,
            "B": B,
            "iota": iota,
        })

    meta = {
        "caps": caps,
        "off": off,
        "total": total,
        "halves": halves,
    }
    return in_maps, meta


# ---------------------------------------------------------------- device code
def _build_nc(caps_key, caps, off, total, halves):
    nc = bacc.Bacc("TRN2", target_bir_lowering=False, debug=False,
                   num_devices=NCORES)

    t_x = nc.dram_tensor("x_aug", [NAUG, D], FP, kind="ExternalInput").ap()
    t_idx = nc.dram_tensor("idxs", [128, total // 16], mybir.dt.int16,
                           kind="ExternalInput").ap()
    t_dl = nc.dram_tensor("dl", [128, total // 128], FP,
                          kind="ExternalInput").ap()
    t_xT = nc.dram_tensor("xT", [128, 2, SHARD_PAD], FP,
                          kind="ExternalInput").ap()
    t_C = nc.dram_tensor("C", [NW, SHARD_PAD], FP, kind="ExternalInput").ap()
    t_Wt = nc.dram_tensor("Wt", [128, 2, NW, D], FP, kind="ExternalInput").ap()
    t_B = nc.dram_tensor("B", [NW, D], FP, kind="ExternalInput").ap()
    t_iota = nc.dram_tensor("iota", [128, BLK], FP, kind="ExternalInput").ap()
    t_out = nc.dram_tensor("outT", [D, SHARD_PAD], FP,
                           kind="ExternalOutput").ap()

    cap_h_max = max(cap_h for (_, _, _, cap_h) in halves)
    halves_by_block = {}
    for (b, h, coff, cap_h) in halves:
        halves_by_block.setdefault(b, []).append((h, coff, cap_h))

    with tile.TileContext(nc) as tc:
        with tc.tile_pool(name="const", bufs=1) as cpool, \
             tc.tile_pool(name="gather", bufs=2) as gpool, \
             tc.tile_pool(name="pbuf", bufs=4) as ppool, \
             tc.tile_pool(name="ssb", bufs=4) as sspool, \
             tc.tile_pool(name="res", bufs=4) as rpool, \
             tc.tile_pool(name="psum_s", bufs=2, space="PSUM") as pspool, \
             tc.tile_pool(name="psum_o", bufs=2, space="PSUM") as popool:

            nc.gpsimd.load_library(library_config.mlp)

            idx_t = cpool.tile([128, total // 16], mybir.dt.int16)
            dl_t = cpool.tile([128, total // 128], FP)
            xT_t = cpool.tile([128, 2, SHARD_PAD], FP)
            C_t = cpool.tile([NW, SHARD_PAD], FP)
            Wt_t = cpool.tile([128, 2, NW, D], FP)
            B_t = cpool.tile([NW, D], FP)
            iota_t = cpool.tile([128, BLK], FP)
            nc.sync.dma_start(out=idx_t[:], in_=t_idx)
            nc.sync.dma_start(out=dl_t[:], in_=t_dl)
            nc.sync.dma_start(out=xT_t[:], in_=t_xT)
            nc.sync.dma_start(out=C_t[:], in_=t_C)
            nc.sync.dma_start(out=Wt_t[:], in_=t_Wt)
            nc.sync.dma_start(out=B_t[:], in_=t_B)
            nc.sync.dma_start(out=iota_t[:], in_=t_iota)

            for b in range(NBLK):
                bsl = slice(b * BLK, (b + 1) * BLK)
                # -------- gather this block's edge sources (two rel-halves)
                gt = {}
                for (h, coff, cap_h) in halves_by_block[b]:
                    g = gpool.tile([128, cap_h_max // 128, D], FP, tag="g")
                    if cap_h > 0:
                        nc.gpsimd.dma_gather(
                            g[:, :cap_h // 128, :], t_x,
                            idx_t[:, coff // 16:(coff + cap_h) // 16],
                            cap_h, cap_h, D, single_packet=False)
                    gt[h] = (g, coff)

                # -------- out_T accumulation
                out_ps = [popool.tile([128, BLK], FP, space="PSUM",
                                      tag=f"out{oi}", name=f"out{oi}") for oi in range(2)]
                nz_rels = [r for r in range(R2) if caps[b][r] > 0]
                # how many matmuls will hit each out_ps tile
                mm_left = [2 + 1 + 2 * len(nz_rels) for _ in range(2)]

                def out_mm(oi, lhsT, rhs, first):
                    mm_left[oi] -= 1
                    nc.tensor.matmul(out=out_ps[oi][:], lhsT=lhsT, rhs=rhs,
                                     start=first, stop=(mm_left[oi] == 0))

                for oi in range(2):
                    osl = slice(oi * 128, (oi + 1) * 128)
                    for fi in range(2):
                        out_mm(oi, Wt_t[:, fi, R2, osl], xT_t[:, fi, bsl],
                               first=(fi == 0))
                    out_mm(oi, B_t[:NW, osl], C_t[:NW, bsl], first=False)

                # -------- per-relation aggregation + transform
                for r in nz_rels:
                    h = 0 if r < R2 // 2 else 1
                    g, coff = gt[h]
                    nch = caps[b][r] // 128
                    roff = int(off[b][r])
                    s_ps = [pspool.tile([128, BLK], FP, space="PSUM",
                                        tag=f"s{fi}", name=f"s{fi}") for fi in range(2)]
                    for ch in range(nch):
                        col = roff // 128 + ch
                        p_t = ppool.tile([128, BLK], FP, tag="p")
                        nc.vector.tensor_scalar(
                            out=p_t[:], in0=iota_t[:],
                            scalar1=dl_t[:, col:col + 1], scalar2=None,
                            op0=mybir.AluOpType.is_equal)
                        gch = (roff - coff) // 128 + ch
                        for fi in range(2):
                            nc.tensor.matmul(
                                out=s_ps[fi][:],
                                lhsT=g[:, gch, fi * 128:(fi + 1) * 128],
                                rhs=p_t[:],
                                start=(ch == 0), stop=(ch == nch - 1))
                    s_sb = [sspool.tile([128, BLK], FP, tag=f"ss{fi}", name=f"ssb{fi}")
                            for fi in range(2)]
                    nc.vector.tensor_copy(out=s_sb[0][:], in_=s_ps[0][:])
                    nc.scalar.copy(out=s_sb[1][:], in_=s_ps[1][:])
                    for oi in range(2):
                        osl = slice(oi * 128, (oi + 1) * 128)
                        for fi in range(2):
                            out_mm(oi, Wt_t[:, fi, r, osl], s_sb[fi][:],
                                   first=False)

                # -------- relu + writeback
                for oi in range(2):
                    osl = slice(oi * 128, (oi + 1) * 128)
                    res = rpool.tile([128, BLK], FP, tag="res")
                    nc.scalar.activation(out=res[:], in_=out_ps[oi][:],
                                         func=mybir.ActivationFunctionType.Relu)
                    nc.sync.dma_start(out=t_out[osl, bsl], in_=res[:])

    nc.compile()
    return nc


def _get_nc(meta):
    key = meta["caps"].tobytes()
    if key not in _CACHE:
        _CACHE[key] = _build_nc(key, meta["caps"], meta["off"],
                                meta["total"], meta["halves"])
    return _CACHE[key]


# ---------------------------------------------------------------- entry point
def kernel(x, dep_idx, rel_idx, gov_idx, W_self, b_self, W_rel, b_rel):
    in_maps, meta = _preprocess(x, dep_idx, rel_idx, gov_idx,
                                W_self, b_self, W_rel, b_rel)
    nc = _get_nc(meta)
    r = run_bass_kernel_spmd(nc, in_maps, core_ids=list(range(NCORES)),
                             trace=False)
    out = np.empty((N, D), dtype=np.float32)
    for c in range(NCORES):
        out[c * SHARD:(c + 1) * SHARD] = r.results[c]["outT"][:, :SHARD].T
    return out


# revision 6
# speedup vs baseline: 3358.8223x; 3358.8223x over previous
"""Trainium2 Bass kernel for nn_Dependency_GCNLayer (relational GCN layer).

Reference semantics:
    out = relu(x @ W_self.T + b_self
               + scatter_add over forward edges:  dep += W_rel[r]   @ x[gov] + b_rel[r]
               + scatter_add over reverse edges:  gov += W_rel[r+R] @ x[dep] + b_rel[r+R])

Strategy (8 cores, SPMD single program):
  - Output rows (nodes) sharded 2500/core. x + weights replicated.
  - Unified edge list (dest, src, rel) with reverse edges mapped to rel+R.
  - Host groups edges by (dest-block of 256 nodes, rel), pads each group to a
    multiple of 128 slots (shared caps across cores => identical instruction
    stream; per-core data differs only through input tensors).
  - Device, per dest-block:
      * dma_gather x[src] rows for the block's edge slots (SWDGE custom op)
      * per rel: one-hot aggregation  S_T[f, d] += G_chunk[e, f].T @ P[e, d]
        (P built on DVE via iota==dest_local), accumulated in PSUM
      * copy S_T -> SBUF, then dense transform out_T[o, d] += W_r.T[f, o].T @ S_T
        accumulated in PSUM across all rels + self term + (counts @ biases)
      * ReLU -> out_T in DRAM, host transposes/concats shards.
"""

import math

import numpy as np

import concourse.bacc as bacc
import concourse.mybir as mybir
import concourse.tile as tile
from concourse import library_config
from concourse.bass_utils import run_bass_kernel_spmd

# ---------------------------------------------------------------- constants
N, D, R, E = 20000, 256, 12, 200000
NCORES = 8
SHARD = N // NCORES          # 2500
BLK = 256                    # dest-node block width (= matmul N)
NBLK = math.ceil(SHARD / BLK)   # 10
SHARD_PAD = NBLK * BLK       # 2560
R2 = 2 * R                   # 24 relation slots (fwd + rev)
NW = R2 + 1                  # + self
NAUG = 20480                 # x rows padded with zero rows (int16-indexable)
ZROW = N                     # first zero row index
FP = mybir.dt.float32

_CACHE = {}


# ---------------------------------------------------------------- host prep
def _wrap_idx(idx_local: np.ndarray) -> np.ndarray:
    """Layout gather indices for dma_gather: slot s -> wrapped[ch, k] with
    ch = s % 16, k = (s // 128) * 8 + (s % 128) // 16 (measured on HW)."""
    n = idx_local.shape[0]
    w = np.zeros((16, n // 16), dtype=np.int16)
    s = np.arange(n)
    w[s % 16, (s // 128) * 8 + (s % 128) // 16] = idx_local.astype(np.int16)
    return np.tile(w, (8, 1))  # [128, n/16]


def _preprocess(x, dep_idx, rel_idx, gov_idx, W_self, b_self, W_rel, b_rel):
    dep = np.asarray(dep_idx).astype(np.int64)
    gov = np.asarray(gov_idx).astype(np.int64)
    rel = np.asarray(rel_idx).astype(np.int64)

    dest = np.concatenate([dep, gov])
    src = np.concatenate([gov, dep])
    rr = np.concatenate([rel, rel + R])

    core = dest // SHARD
    loc = dest % SHARD
    blk = loc // BLK
    dl = (loc % BLK).astype(np.float32)

    gid = (core * NBLK + blk) * R2 + rr           # group id
    counts = np.bincount(gid, minlength=NCORES * NBLK * R2)
    counts = counts.reshape(NCORES, NBLK, R2)
    caps = counts.max(axis=0)                     # [NBLK, R2]
    caps = ((caps + 127) // 128) * 128

    # slot offsets per (blk, rel), shared by all cores
    off = np.zeros((NBLK, R2), dtype=np.int64)
    off.flat[1:] = np.cumsum(caps.flat)[:-1]
    total = int(caps.sum())

    # rank of each edge within its (core, blk, rel) group
    order = np.argsort(gid, kind="stable")
    gs = gid[order]
    starts = np.searchsorted(gs, np.arange(NCORES * NBLK * R2))
    rank = np.arange(gs.shape[0]) - starts[gs]
    slot = off[blk[order], rr[order]] + rank      # slot within core's array
    core_o = core[order]

    idx_arr = np.full((NCORES, total), ZROW, dtype=np.int64)
    dl_arr = np.full((NCORES, total), -1.0, dtype=np.float32)
    idx_arr[core_o, slot] = src[order]
    dl_arr[core_o, slot] = dl[order]

    # gather calls: per (block, rel-half); all slot ranges 128-aligned
    halves = []
    for b in range(NBLK):
        for h in range(2):
            r0, r1 = h * (R2 // 2), (h + 1) * (R2 // 2)
            coff = int(off[b, r0])
            cap_h = int(caps[b, r0:r1].sum())
            halves.append((b, h, coff, cap_h))

    x = np.asarray(x, np.float32)
    x_aug = np.zeros((NAUG, D), dtype=np.float32)
    x_aug[:N] = x

    # per-core tensors
    W_rel = np.asarray(W_rel, np.float32)
    W_self = np.asarray(W_self, np.float32)
    b_rel = np.asarray(b_rel, np.float32)
    b_self = np.asarray(b_self, np.float32)

    Wt = np.zeros((128, 2, NW, D), dtype=np.float32)
    Wrt = W_rel.transpose(2, 0, 1).reshape(2, 128, R2, D)   # [fi, p, r, o]
    Wt[:, :, :R2, :] = Wrt.transpose(1, 0, 2, 3)
    Wst = W_self.T.reshape(2, 128, D)                       # [fi, p, o]
    Wt[:, :, R2, :] = Wst.transpose(1, 0, 2)

    B = np.zeros((NW, D), dtype=np.float32)
    B[:R2] = b_rel
    B[R2] = b_self

    iota = np.broadcast_to(np.arange(BLK, dtype=np.float32), (128, BLK)).copy()

    in_maps = []
    for c in range(NCORES):
        xs = x[c * SHARD:(c + 1) * SHARD]
        xT = np.zeros((128, 2, SHARD_PAD), dtype=np.float32)
        xT[:, :, :SHARD] = xs.T.reshape(2, 128, SHARD).transpose(1, 0, 2)

        lc = loc[core == c]
        rc = rr[core == c]
        C = np.zeros((NW, SHARD_PAD), dtype=np.float32)
        cc = np.bincount(lc * R2 + rc, minlength=SHARD * R2).reshape(SHARD, R2)
        C[:R2, :SHARD] = cc.T
        C[R2, :SHARD] = 1.0

        wrapped = np.concatenate(
            [_wrap_idx(idx_arr[c, coff:coff + cap_h])
             for (_, _, coff, cap_h) in halves], axis=1)
        dlt = dl_arr[c].reshape(-1, 128).T.copy()            # [128, total/128]

        in_maps.append({
            "x_aug": x_aug,
            "idxs": wrapped,
            "dl": dlt,
            "xT": xT,
            "C": C,
            "Wt": Wt,
            "B": B,
            "iota": iota,
        })

    meta = {
        "caps": caps,
        "off": off,
        "total": total,
        "halves": halves,
    }
    return in_maps, meta


# ---------------------------------------------------------------- device code
def _build_nc(caps, off, total, halves, repeat=1):
    nc = bacc.Bacc("TRN2", target_bir_lowering=False, debug=False,
                   num_devices=NCORES)

    t_x = nc.dram_tensor("x_aug", [NAUG, D], FP, kind="ExternalInput").ap()
    t_idx = nc.dram_tensor("idxs", [128, total // 16], mybir.dt.int16,
                           kind="ExternalInput").ap()
    t_dl = nc.dram_tensor("dl", [128, total // 128], FP,
                          kind="ExternalInput").ap()
    t_xT = nc.dram_tensor("xT", [128, 2, SHARD_PAD], FP,
                          kind="ExternalInput").ap()
    t_C = nc.dram_tensor("C", [NW, SHARD_PAD], FP, kind="ExternalInput").ap()
    t_Wt = nc.dram_tensor("Wt", [128, 2, NW, D], FP, kind="ExternalInput").ap()
    t_B = nc.dram_tensor("B", [NW, D], FP, kind="ExternalInput").ap()
    t_iota = nc.dram_tensor("iota", [128, BLK], FP, kind="ExternalInput").ap()
    t_out = nc.dram_tensor("outT", [D, SHARD_PAD], FP,
                           kind="ExternalOutput").ap()

    cap_h_max = max(cap_h for (_, _, _, cap_h) in halves)
    halves_by_block = {}
    for (b, h, coff, cap_h) in halves:
        halves_by_block.setdefault(b, []).append((h, coff, cap_h))

    with tile.TileContext(nc) as tc:
        with tc.tile_pool(name="const", bufs=1) as cpool, \
             tc.tile_pool(name="gather", bufs=2) as gpool, \
             tc.tile_pool(name="pbuf", bufs=4) as ppool, \
             tc.tile_pool(name="ssb", bufs=4) as sspool, \
             tc.tile_pool(name="res", bufs=4) as rpool, \
             tc.tile_pool(name="psum_s", bufs=2, space="PSUM") as pspool, \
             tc.tile_pool(name="psum_o", bufs=2, space="PSUM") as popool:

            nc.gpsimd.load_library(library_config.mlp)

            idx_t = cpool.tile([128, total // 16], mybir.dt.int16)
            dl_t = cpool.tile([128, total // 128], FP)
            xT_t = cpool.tile([128, 2, SHARD_PAD], FP)
            C_t = cpool.tile([NW, SHARD_PAD], FP)
            Wt_t = cpool.tile([128, 2, NW, D], FP)
            B_t = cpool.tile([NW, D], FP)
            iota_t = cpool.tile([128, BLK], FP)
            nc.sync.dma_start(out=idx_t[:], in_=t_idx)
            nc.sync.dma_start(out=dl_t[:], in_=t_dl)
            nc.sync.dma_start(out=xT_t[:], in_=t_xT)
            nc.sync.dma_start(out=C_t[:], in_=t_C)
            nc.sync.dma_start(out=Wt_t[:], in_=t_Wt)
            nc.sync.dma_start(out=B_t[:], in_=t_B)
            nc.sync.dma_start(out=iota_t[:], in_=t_iota)

            def body():
                for b in range(NBLK):
                    bsl = slice(b * BLK, (b + 1) * BLK)
                    # ---- gather this block's edge sources (two rel-halves)
                    gt = {}
                    for (h, coff, cap_h) in halves_by_block[b]:
                        g = gpool.tile([128, cap_h_max // 128, D], FP,
                                       tag="g", name="g")
                        if cap_h > 0:
                            nc.gpsimd.dma_gather(
                                g[:, :cap_h // 128, :], t_x,
                                idx_t[:, coff // 16:(coff + cap_h) // 16],
                                cap_h, cap_h, D, single_packet=False)
                        gt[h] = (g, coff)

                    # ---- out_T accumulation
                    out_ps = [popool.tile([128, BLK], FP, space="PSUM",
                                          tag=f"out{oi}", name=f"out{oi}")
                              for oi in range(2)]
                    nz_rels = [r for r in range(R2) if caps[b][r] > 0]
                    mm_left = [2 + 1 + 2 * len(nz_rels) for _ in range(2)]

                    def out_mm(oi, lhsT, rhs, first):
                        mm_left[oi] -= 1
                        nc.tensor.matmul(out=out_ps[oi][:], lhsT=lhsT,
                                         rhs=rhs, start=first,
                                         stop=(mm_left[oi] == 0))

                    for oi in range(2):
                        osl = slice(oi * 128, (oi + 1) * 128)
                        for fi in range(2):
                            out_mm(oi, Wt_t[:, fi, R2, osl],
                                   xT_t[:, fi, bsl], first=(fi == 0))
                        out_mm(oi, B_t[:NW, osl], C_t[:NW, bsl], first=False)

                    # ---- per-relation aggregation + transform
                    for r in nz_rels:
                        h = 0 if r < R2 // 2 else 1
                        g, coff = gt[h]
                        nch = caps[b][r] // 128
                        roff = int(off[b][r])
                        s_ps = [pspool.tile([128, BLK], FP, space="PSUM",
                                            tag=f"s{fi}", name=f"s{fi}")
                                for fi in range(2)]
                        for ch in range(nch):
                            col = roff // 128 + ch
                            p_t = ppool.tile([128, BLK], FP, tag="p", name="p")
                            nc.vector.tensor_scalar(
                                out=p_t[:], in0=iota_t[:],
                                scalar1=dl_t[:, col:col + 1], scalar2=None,
                                op0=mybir.AluOpType.is_equal)
                            gch = (roff - coff) // 128 + ch
                            for fi in range(2):
                                nc.tensor.matmul(
                                    out=s_ps[fi][:],
                                    lhsT=g[:, gch, fi * 128:(fi + 1) * 128],
                                    rhs=p_t[:],
                                    start=(ch == 0), stop=(ch == nch - 1))
                        s_sb = [sspool.tile([128, BLK], FP, tag=f"ss{fi}",
                                            name=f"ssb{fi}")
                                for fi in range(2)]
                        nc.vector.tensor_copy(out=s_sb[0][:], in_=s_ps[0][:])
                        nc.scalar.copy(out=s_sb[1][:], in_=s_ps[1][:])
                        for oi in range(2):
                            osl = slice(oi * 128, (oi + 1) * 128)
                            for fi in range(2):
                                out_mm(oi, Wt_t[:, fi, r, osl], s_sb[fi][:],
                                       first=False)

                    # ---- relu + writeback
                    for oi in range(2):
                        osl = slice(oi * 128, (oi + 1) * 128)
                        res = rpool.tile([128, BLK], FP, tag="res", name="res")
                        nc.scalar.activation(
                            out=res[:], in_=out_ps[oi][:],
                            func=mybir.ActivationFunctionType.Relu)
                        nc.sync.dma_start(out=t_out[osl, bsl], in_=res[:])

            if repeat > 1:
                with tc.For_i(0, repeat, 1):
                    body()
            else:
                body()

    nc.compile()
    return nc


def _get_nc(meta, repeat=1):
    key = (meta["caps"].tobytes(), repeat)
    if key not in _CACHE:
        _CACHE[key] = _build_nc(meta["caps"], meta["off"], meta["total"],
                                meta["halves"], repeat=repeat)
    return _CACHE[key]


# ---------------------------------------------------------------- entry point
def kernel(x, dep_idx, rel_idx, gov_idx, W_self, b_self, W_rel, b_rel):
    in_maps, meta = _preprocess(x, dep_idx, rel_idx, gov_idx,
                                W_self, b_self, W_rel, b_rel)
    nc = _get_nc(meta)
    r = run_bass_kernel_spmd(nc, in_maps, core_ids=list(range(NCORES)),
                             trace=False)
    out = np.empty((N, D), dtype=np.float32)
    for c in range(NCORES):
        out[c * SHARD:(c + 1) * SHARD] = r.results[c]["outT"][:, :SHARD].T
    return out
